# revision 32
# baseline (speedup 1.0000x reference)
"""Bass/Tile kernel for nn_MicrotubuleAttention on 8 Trainium2 NeuronCores.

Math: the reference adds (1 - gtp) * NEG (NEG = -1e9) to every causal
off-diagonal score. With gamma clipped to >= 1e-4, the smallest penalty is
-1e9 * (1 - exp(-1e-4)) ~= -1e5, so after float32 softmax (max-subtract +
exp) every off-diagonal weight underflows to exactly 0 and attention is
exactly the identity. Hence:

    out = repeat_gqa(x @ Wv) @ Wo = (x @ Wv) @ Wo_folded

where Wo_folded[c*64+d, :] = sum_r Wo[(4c+r)*64+d, :] sums the 4 query-head
row blocks that share KV head c. Q/K/RoPE/polarity/gamma provably do not
affect the f32 output (verified ~1e-6 max rel err against the jax reference).

Sharding: data parallel over rows. B*T = 4096 rows split 8 ways -> 512 rows
per core; Wv/Wo broadcast. Per core, pipelined per 128-row chunk mi:
  1. transpose chunk mi via PE (fp32 has no DMA transpose)
  2. stage 1 for chunk mi:  vT[j', mi] = Wv[:, j']^T @ xT[:, :, mi]
  3. (in parallel) fold Wo: pair adds on DVE, 64-partition shift via
     4 independent SBUF->SBUF DMAs, final adds on DVE -> WoF [256, 1024]
  4. stage 2:  out[mi, n] = vT[:, mi]^T @ WoF

DMA priority: x gates the PE pipeline, Wv gates stage 1, Wo only gates
stage 2, so the Wo loads are chained behind the last x chunk.
"""

import os
import sys

import numpy as np

for _p in ("/opt/trn_rl_repo", "/opt/pypackages"):
    if os.path.isdir(_p) and _p not in sys.path:
        sys.path.append(_p)

B, T, D_MODEL = 2, 2048, 1024
H_Q, H_KV, D_HEAD = 16, 4, 64
N_CORES = 8
M_TOTAL = B * T              # 4096 rows
M_CORE = M_TOTAL // N_CORES  # 512 rows per core
P = 128
KK = D_MODEL // P            # 8 contraction chunks of 128
MC = M_CORE // P             # 4 row chunks of 128
NKV = H_KV * D_HEAD          # 256

TRACE = False          # test.py flips this to profile
TRACE_CORES = None
LAST_RESULTS = None    # BassKernelResults of the most recent run

_nc_cache = None


def _build_bass():
    import concourse.bass as bass
    import concourse.mybir as mybir
    import concourse.tile as tile
    from concourse import bacc
    from concourse.masks import make_identity
    from concourse.tile import add_dep_helper

    f32 = mybir.dt.float32
    ts = bass.ts

    nc = bacc.Bacc(None)
    x_d = nc.declare_dram_parameter("x", [M_CORE, D_MODEL], f32, isOutput=False)
    wv_d = nc.declare_dram_parameter("wv", [D_MODEL, NKV], f32, isOutput=False)
    wo_d = nc.declare_dram_parameter("wo", [H_Q * D_HEAD, D_MODEL], f32, isOutput=False)
    out_d = nc.declare_dram_parameter("out", [M_CORE, D_MODEL], f32, isOutput=True)

    with tile.TileContext(nc) as tc:
        with (
            tc.tile_pool(name="const", bufs=1) as const,
            tc.tile_pool(name="wo_pool", bufs=H_KV) as wo_pool,
            tc.tile_pool(name="x_pool", bufs=MC) as x_pool,
            tc.tile_pool(name="o_pool", bufs=2 * MC) as o_pool,
            tc.tile_pool(name="psum_t", bufs=3, space="PSUM") as psum_t,
            tc.tile_pool(name="psum_s1", bufs=2, space="PSUM") as psum_s1,
            tc.tile_pool(name="psum_s2", bufs=3, space="PSUM") as psum_s2,
        ):
            identity = const.tile([P, P], f32)
            make_identity(nc, identity)
            warm = psum_t.tile([P, 512], f32, tag="tp")
            for _ in range(6):
                nc.tensor.transpose(warm[:, :P], identity[:], identity[:])

            wv_sb = const.tile([P, KK, NKV], f32)
            xT = const.tile([P, KK, M_CORE], f32)
            vT = const.tile([P, 2, M_CORE], f32)

            # ---- per-chunk pipeline: transposes run PIPE chunks ahead of
            # stage 1 so early stage-1 work never stalls on the (slow,
            # strided) Wv load; x halves load separately for finer arrival.
            def emit_transpose(mi, x_sb, g):
                pt = psum_t.tile([P, 512], f32, tag="tp")
                for j in range(4):
                    kk = g * 4 + j
                    nc.tensor.transpose(
                        pt[:, ts(j, P)], x_sb[:, g, ts(j, P)], identity[:]
                    )
                nc.scalar.copy(
                    xT[:, ts(g, 4), ts(mi, P)],
                    pt.rearrange("p (j m) -> p j m", j=4),
                )

            # stage 1 split at the contraction midpoint: kk 0-3 only needs
            # the g0 transposes, so those matmuls fill the PE's wait for the
            # g1 x-halves (open PSUM accumulation groups interleave fine).
            s1_ps = {}

            def emit_stage1(q, half):
                if half == 0:
                    s1_ps[q] = psum_s1.tile([P, M_CORE], f32, tag="s1", name=f"s1_{q}")
                ps = s1_ps[q]
                for kk in range(half * 4, half * 4 + 4):
                    nc.tensor.matmul(
                        ps[:],
                        lhsT=wv_sb[:, kk, ts(q, P)],
                        rhs=xT[:, kk, :],
                        start=(kk == 0),
                        stop=(kk == KK - 1),
                    )
                if half == 1:
                    nc.vector.tensor_copy(vT[:, q, :], ps[:])

            # Wo loads split over both HWDGE queues, emitted after the last
            # x DMAs on each queue; gated behind x chunk 1 so they never
            # steal HBM bandwidth from the PE-feeding x stream. The fold
            # adds are emitted right after stage1(m0) so the in-order DVE
            # queue reaches them as soon as the Wo data lands (they were
            # stuck behind all stage-1 copybacks before).
            wo_f = const.tile([P, 2, D_MODEL], f32)

            def emit_wo_loads():
                loads = []
                for c in range(H_KV):
                    t01 = wo_pool.tile([P, 2, D_MODEL], f32, tag="wo_raw")
                    loads.append(
                        (
                            t01,
                            nc.gpsimd.dma_start(
                                t01[:],
                                wo_d[256 * c : 256 * (c + 1), :].rearrange(
                                    "(two p) n -> p two n", p=P
                                ),
                            ),
                        )
                    )
                return loads

            def emit_wo_folds(loads):
                for c, (t01, _) in enumerate(loads):
                    pair = wo_pool.tile([P, D_MODEL], f32, tag="wo_pair")
                    nc.vector.tensor_add(pair[:], t01[:, 0, :], t01[:, 1, :])
                    shift = wo_pool.tile([64, D_MODEL], f32, tag="wo_shift")
                    nc.gpsimd.dma_start(shift[:], pair[64:128, :])
                    lo = (c % 2) * 64
                    nc.vector.tensor_add(
                        wo_f[lo : lo + 64, c // 2, :], pair[0:64, :], shift[:]
                    )

            # Each chunk loads as two half-DMAs issued in parallel on both
            # HWDGE queues (per-DMA queue bandwidth ~200GB/s was the arrival
            # bottleneck); transposes of group g wait only on half g.
            PIPE = 3
            x_dmas = []
            wo_loads = None
            xv = x_d.rearrange("m (g n) -> m g n", g=2)
            x_sbs = []
            for mi in range(MC):
                x_sb = x_pool.tile([P, 2, 512], f32, tag="x_in")
                x_sbs.append(x_sb)
                x_dmas.append(nc.sync.dma_start(x_sb[:, 0, :], xv[ts(mi, P), 0, :]))
                x_dmas.append(nc.scalar.dma_start(x_sb[:, 1, :], xv[ts(mi, P), 1, :]))
                if mi == 0:
                    # Wv: strided 1KB-descriptor layout is descriptor-rate
                    # bound; 4 parallel SWDGE quarter-loads spread it over
                    # more SDMA queues.
                    wv_v = wv_d.rearrange("(ko p) n -> ko p n", p=P)
                    for ko in range(0, KK, 2):
                        nc.gpsimd.dma_start(
                            wv_sb[:, ko : ko + 2, :],
                            wv_v[ko : ko + 2].rearrange("ko p n -> p ko n"),
                        )
                if mi == MC - 1:
                    wo_loads = emit_wo_loads()
                emit_transpose(mi, x_sb, 0)
                if mi == MC - 1:
                    emit_wo_folds(wo_loads)
            for q in range(2):
                emit_stage1(q, 0)
            for mi in range(MC):
                emit_transpose(mi, x_sbs[mi], 1)
            for q in range(2):
                emit_stage1(q, 1)
            for _, d0 in wo_loads[:2]:
                add_dep_helper(d0.ins, x_dmas[2].ins, reason="x1 before wo")


            # ---- stage 2: out[m, n] = sum_j' vT[j', m] WoF[j', n] ----
            for mi in range(MC):
                for half in range(2):
                    ps = psum_s2.tile([P, 512], f32, tag="s2")
                    for q in range(2):
                        nc.tensor.matmul(
                            ps[:],
                            lhsT=vT[:, q, ts(mi, P)],
                            rhs=wo_f[:, q, ts(half, 512)],
                            start=(q == 0),
                            stop=(q == 1),
                        )
                    o_sb = o_pool.tile([P, 512], f32, tag="o_sb")
                    nc.vector.tensor_copy(o_sb[:], ps[:])
                    eng_o = nc.sync if half == 0 else nc.scalar
                    eng_o.dma_start(out_d[ts(mi, P), ts(half, 512)], o_sb[:])

    nc.finalize()
    return nc


def _get_nc():
    global _nc_cache
    if _nc_cache is None:
        _nc_cache = _build_bass()
    return _nc_cache


def kernel(**inputs) -> np.ndarray:
    global LAST_RESULTS
    from concourse.bass_utils import run_bass_kernel_spmd

    x = np.ascontiguousarray(
        np.asarray(inputs["x"], dtype=np.float32).reshape(M_TOTAL, D_MODEL)
    )
    wv = np.ascontiguousarray(np.asarray(inputs["Wv"], dtype=np.float32))
    wo = np.ascontiguousarray(np.asarray(inputs["Wo"], dtype=np.float32))

    nc = _get_nc()
    in_maps = [
        {"x": x[i * M_CORE : (i + 1) * M_CORE], "wv": wv, "wo": wo}
        for i in range(N_CORES)
    ]
    res = run_bass_kernel_spmd(
        nc,
        in_maps,
        list(range(N_CORES)),
        trace=TRACE,
        trace_cores=TRACE_CORES,
    )
    LAST_RESULTS = res
    out = np.concatenate([r["out"] for r in res.results], axis=0)
    return out.reshape(B, T, D_MODEL)


# revision 33
# speedup vs baseline: 1.0059x; 1.0059x over previous
"""Bass/Tile kernel for nn_MicrotubuleAttention on 8 Trainium2 NeuronCores.

Math: the reference adds (1 - gtp) * NEG (NEG = -1e9) to every causal
off-diagonal score. With gamma clipped to >= 1e-4, the smallest penalty is
-1e9 * (1 - exp(-1e-4)) ~= -1e5, so after float32 softmax (max-subtract +
exp) every off-diagonal weight underflows to exactly 0 and attention is
exactly the identity. Hence:

    out = repeat_gqa(x @ Wv) @ Wo = (x @ Wv) @ Wo_folded

where Wo_folded[c*64+d, :] = sum_r Wo[(4c+r)*64+d, :] sums the 4 query-head
row blocks that share KV head c. Q/K/RoPE/polarity/gamma provably do not
affect the f32 output (verified ~1e-6 max rel err against the jax reference).

Sharding: data parallel over rows. B*T = 4096 rows split 8 ways -> 512 rows
per core; Wv/Wo broadcast. Per core, pipelined per 128-row chunk mi:
  1. transpose chunk mi via PE (fp32 has no DMA transpose)
  2. stage 1 for chunk mi:  vT[j', mi] = Wv[:, j']^T @ xT[:, :, mi]
  3. (in parallel) fold Wo: pair adds on DVE, 64-partition shift via
     4 independent SBUF->SBUF DMAs, final adds on DVE -> WoF [256, 1024]
  4. stage 2:  out[mi, n] = vT[:, mi]^T @ WoF

DMA priority: x gates the PE pipeline, Wv gates stage 1, Wo only gates
stage 2, so the Wo loads are chained behind the last x chunk.
"""

import os
import sys

import numpy as np

for _p in ("/opt/trn_rl_repo", "/opt/pypackages"):
    if os.path.isdir(_p) and _p not in sys.path:
        sys.path.append(_p)

B, T, D_MODEL = 2, 2048, 1024
H_Q, H_KV, D_HEAD = 16, 4, 64
N_CORES = 8
M_TOTAL = B * T              # 4096 rows
M_CORE = M_TOTAL // N_CORES  # 512 rows per core
P = 128
KK = D_MODEL // P            # 8 contraction chunks of 128
MC = M_CORE // P             # 4 row chunks of 128
NKV = H_KV * D_HEAD          # 256

TRACE = False          # test.py flips this to profile
TRACE_CORES = None
LAST_RESULTS = None    # BassKernelResults of the most recent run

_nc_cache = None


def _build_bass():
    import concourse.bass as bass
    import concourse.mybir as mybir
    import concourse.tile as tile
    from concourse import bacc
    from concourse.masks import make_identity
    from concourse.tile import add_dep_helper

    f32 = mybir.dt.float32
    ts = bass.ts

    nc = bacc.Bacc(None)
    x_d = nc.declare_dram_parameter("x", [M_CORE, D_MODEL], f32, isOutput=False)
    wv_d = nc.declare_dram_parameter("wv", [D_MODEL, NKV], f32, isOutput=False)
    wo_d = nc.declare_dram_parameter("wo", [H_Q * D_HEAD, D_MODEL], f32, isOutput=False)
    out_d = nc.declare_dram_parameter("out", [M_CORE, D_MODEL], f32, isOutput=True)

    with tile.TileContext(nc) as tc:
        with (
            tc.tile_pool(name="const", bufs=1) as const,
            tc.tile_pool(name="wo_pool", bufs=H_KV) as wo_pool,
            tc.tile_pool(name="x_pool", bufs=MC) as x_pool,
            tc.tile_pool(name="o_pool", bufs=2 * MC) as o_pool,
            tc.tile_pool(name="psum_t", bufs=3, space="PSUM") as psum_t,
            tc.tile_pool(name="psum_s1", bufs=2, space="PSUM") as psum_s1,
            tc.tile_pool(name="psum_s2", bufs=3, space="PSUM") as psum_s2,
        ):
            identity = const.tile([P, P], f32)
            make_identity(nc, identity)
            warm = psum_t.tile([P, 512], f32, tag="tp")
            nc.tensor.transpose(warm[:, :P], identity[:], identity[:])

            wv_sb = const.tile([P, KK, NKV], f32)
            xT = const.tile([P, KK, M_CORE], f32)
            vT = const.tile([P, 2, M_CORE], f32)

            # ---- per-chunk pipeline: transposes run PIPE chunks ahead of
            # stage 1 so early stage-1 work never stalls on the (slow,
            # strided) Wv load; x halves load separately for finer arrival.
            def emit_transpose(mi, x_sb):
                for g in range(2):
                    pt = psum_t.tile([P, 512], f32, tag="tp")
                    for j in range(4):
                        kk = g * 4 + j
                        nc.tensor.transpose(
                            pt[:, ts(j, P)], x_sb[:, g, ts(j, P)], identity[:]
                        )
                    nc.scalar.copy(
                        xT[:, ts(g, 4), ts(mi, P)],
                        pt.rearrange("p (j m) -> p j m", j=4),
                    )

            def emit_stage1(q):
                ps = psum_s1.tile([P, M_CORE], f32, tag="s1")
                for kk in range(KK):
                    nc.tensor.matmul(
                        ps[:],
                        lhsT=wv_sb[:, kk, ts(q, P)],
                        rhs=xT[:, kk, :],
                        start=(kk == 0),
                        stop=(kk == KK - 1),
                    )
                nc.vector.tensor_copy(vT[:, q, :], ps[:])

            # Wo loads split over both HWDGE queues, emitted after the last
            # x DMAs on each queue; gated behind x chunk 1 so they never
            # steal HBM bandwidth from the PE-feeding x stream. The fold
            # adds are emitted right after stage1(m0) so the in-order DVE
            # queue reaches them as soon as the Wo data lands (they were
            # stuck behind all stage-1 copybacks before).
            wo_f = const.tile([P, 2, D_MODEL], f32)

            def emit_wo_loads():
                loads = []
                for c in range(H_KV):
                    t01 = wo_pool.tile([P, 2, D_MODEL], f32, tag="wo_raw")
                    loads.append(
                        (
                            t01,
                            nc.gpsimd.dma_start(
                                t01[:],
                                wo_d[256 * c : 256 * (c + 1), :].rearrange(
                                    "(two p) n -> p two n", p=P
                                ),
                            ),
                        )
                    )
                return loads

            def emit_wo_folds(loads):
                for c, (t01, _) in enumerate(loads):
                    pair = wo_pool.tile([P, D_MODEL], f32, tag="wo_pair")
                    nc.vector.tensor_add(pair[:], t01[:, 0, :], t01[:, 1, :])
                    shift = wo_pool.tile([64, D_MODEL], f32, tag="wo_shift")
                    nc.gpsimd.dma_start(shift[:], pair[64:128, :])
                    lo = (c % 2) * 64
                    nc.vector.tensor_add(
                        wo_f[lo : lo + 64, c // 2, :], pair[0:64, :], shift[:]
                    )

            # Each chunk loads as two half-DMAs issued in parallel on both
            # HWDGE queues (per-DMA queue bandwidth ~200GB/s was the arrival
            # bottleneck); transposes of group g wait only on half g.
            PIPE = 3
            x_dmas = []
            wo_loads = None
            xv = x_d.rearrange("m (g n) -> m g n", g=2)
            for mi in range(MC):
                x_sb = x_pool.tile([P, 2, 512], f32, tag="x_in")
                x_dmas.append(nc.sync.dma_start(x_sb[:, 0, :], xv[ts(mi, P), 0, :]))
                x_dmas.append(nc.scalar.dma_start(x_sb[:, 1, :], xv[ts(mi, P), 1, :]))
                if mi == 0:
                    # Wv: strided 1KB-descriptor layout is descriptor-rate
                    # bound; 4 parallel SWDGE quarter-loads spread it over
                    # more SDMA queues.
                    wv_v = wv_d.rearrange("(ko p) n -> ko p n", p=P)
                    for ko in range(0, KK, 2):
                        nc.gpsimd.dma_start(
                            wv_sb[:, ko : ko + 2, :],
                            wv_v[ko : ko + 2].rearrange("ko p n -> p ko n"),
                        )
                if mi == MC - 1:
                    wo_loads = emit_wo_loads()
                emit_transpose(mi, x_sb)
                if mi == MC - 1:
                    emit_wo_folds(wo_loads)
            for q in range(2):
                emit_stage1(q)
            for _, d0 in wo_loads[:2]:
                add_dep_helper(d0.ins, x_dmas[2].ins, reason="x1 before wo")


            # ---- stage 2: out[m, n] = sum_j' vT[j', m] WoF[j', n] ----
            for mi in range(MC):
                for half in range(2):
                    ps = psum_s2.tile([P, 512], f32, tag="s2")
                    for q in range(2):
                        nc.tensor.matmul(
                            ps[:],
                            lhsT=vT[:, q, ts(mi, P)],
                            rhs=wo_f[:, q, ts(half, 512)],
                            start=(q == 0),
                            stop=(q == 1),
                        )
                    o_sb = o_pool.tile([P, 512], f32, tag="o_sb")
                    nc.vector.tensor_copy(o_sb[:], ps[:])
                    eng_o = nc.sync if half == 0 else nc.scalar
                    eng_o.dma_start(out_d[ts(mi, P), ts(half, 512)], o_sb[:])

    nc.finalize()
    return nc


def _get_nc():
    global _nc_cache
    if _nc_cache is None:
        _nc_cache = _build_bass()
    return _nc_cache


def kernel(**inputs) -> np.ndarray:
    global LAST_RESULTS
    from concourse.bass_utils import run_bass_kernel_spmd

    x = np.ascontiguousarray(
        np.asarray(inputs["x"], dtype=np.float32).reshape(M_TOTAL, D_MODEL)
    )
    wv = np.ascontiguousarray(np.asarray(inputs["Wv"], dtype=np.float32))
    wo = np.ascontiguousarray(np.asarray(inputs["Wo"], dtype=np.float32))

    nc = _get_nc()
    in_maps = [
        {"x": x[i * M_CORE : (i + 1) * M_CORE], "wv": wv, "wo": wo}
        for i in range(N_CORES)
    ]
    res = run_bass_kernel_spmd(
        nc,
        in_maps,
        list(range(N_CORES)),
        trace=TRACE,
        trace_cores=TRACE_CORES,
    )
    LAST_RESULTS = res
    out = np.concatenate([r["out"] for r in res.results], axis=0)
    return out.reshape(B, T, D_MODEL)
